# revision 21
# baseline (speedup 1.0000x reference)
"""Self-contained Trainium2 Bass kernel for a 2-layer GAT (GATConv x2, PyG-style).

Contract: kernel(**inputs) takes the FULL inputs (x [N,128] f32, edge_index
[2,E] int, W1/att_src1/att_dst1/b1/W2/att_src2/att_dst2/b2) and returns the
FULL [N,128] f32 output, distributing work across 8 NeuronCores internally.

Strategy (graph-parallel, destination-sharded):
  - Destinations are sharded across the 8 cores; each core owns 6272 padded
    node positions arranged into 49 blocks of 128 (dst = SBUF partition).
  - Per layer every core redundantly computes h_ext = x @ W_ext for ALL nodes
    (x replicated by the host => no exchange before layer 1) and writes 512B
    bf16 rows [h bf16(128) | pad | a_src f32] to local DRAM.
  - Per block, source features are fetched with dma_gather (int16 indices,
    two 25088-row windows) into [128 dst, S slots, 256] tiles; softmax and the
    weighted sum run on ACT/DVE per partition; no cross-partition reduction.
  - Between layers, one AllGather exchanges [x2^T bf16 | a_src2 f32] shards
    (a_dst2 is recomputed locally from the own shard).
"""

import hashlib
from contextlib import ExitStack

import ml_dtypes
import numpy as np

# ---------------------------------------------------------------------------
# Configuration
# ---------------------------------------------------------------------------

F = 128              # feature dim (all of F_in, H, F_out)
CORES = 8
ROW = 256            # bf16 elements per h_ext row (512 B)
ACOL = 65            # f32 column (within bitcast row) holding a_src
NEG_SLOPE = 0.2
MASK_NEG = -30000.0
GCHUNK = 4           # max gathered slot-columns (128 idxs each) per dma_gather
import os as _os
NQUEUES = int(_os.environ.get("GAT_NQUEUES", "4"))   # SWDGE queues for dma_gather desc-gen
GCHUNK = int(_os.environ.get("GAT_GCHUNK", str(GCHUNK)))


class Cfg:
    def __init__(self, n_nodes, per_core_blocks):
        self.N = n_nodes
        self.NB = per_core_blocks              # blocks of 128 dsts per core
        self.PERP = per_core_blocks * 128      # padded positions per core
        self.PER = n_nodes // CORES            # real nodes per core
        assert self.PER * CORES == n_nodes
        assert self.PERP >= self.PER
        self.NPAD = self.PERP * CORES
        self.WIN = self.PERP * (CORES // 2)    # gather window boundary
        assert self.WIN - 1 < 32768, "window must fit int16"
        self.NT = self.NPAD // 128             # global node tiles


FULL_CFG = Cfg(50000, 49)


# ---------------------------------------------------------------------------
# Host-side topology preprocessing (pure graph structure, no feature math)
# ---------------------------------------------------------------------------

def _snake_order(w0, w1):
    """Order dsts by (w0, w1) with alternating w1 direction per w0-run, so
    consecutive 128-groups have near-constant (w0, w1)."""
    idx = np.lexsort((w1, w0))
    w0s = w0[idx]
    out = []
    i = 0
    flip = False
    # iterate runs of equal w0
    while i < len(idx):
        j = i
        while j < len(idx) and w0s[j] == w0s[i]:
            j += 1
        run = idx[i:j]
        out.append(run[::-1] if flip else run)
        flip = not flip
        i = j
    return np.concatenate(out)


def build_topology(cfg, edge_index):
    """Returns the shared block structure + per-core gather metadata."""
    src = np.asarray(edge_index[0], dtype=np.int64)
    dst = np.asarray(edge_index[1], dtype=np.int64)
    N, PER, PERP, NB = cfg.N, cfg.PER, cfg.PERP, cfg.NB
    half = N // 2  # orig-id window boundary (cores 0-3 vs 4-7)

    src_core = src // PER

    # per-core dst permutations and per-block structure
    per_core = []
    orig2pos = np.full(N, -1, dtype=np.int64)
    for k in range(CORES):
        lo = PER * k
        m = (dst >= lo) & (dst < lo + PER)
        s_k = src[m]
        d_k = dst[m] - lo
        w0 = np.bincount(d_k[s_k < half], minlength=PER)
        w1 = np.bincount(d_k[s_k >= half], minlength=PER)
        order = _snake_order(w0, w1)          # local rank -> orig-local dst
        w0o = np.concatenate([w0[order], np.zeros(PERP - PER, np.int64)])
        w1o = np.concatenate([w1[order], np.zeros(PERP - PER, np.int64)])
        orig2pos[order + lo] = np.arange(PER) + PERP * k
        per_core.append({
            "edges_src": s_k, "edges_dst_local": d_k,
            "S0k": w0o.reshape(NB, 128).max(1),
            "S1k": w1o.reshape(NB, 128).max(1),
        })

    S0 = np.max([c["S0k"] for c in per_core], axis=0).astype(np.int64)
    S1 = np.max([c["S1k"] for c in per_core], axis=0).astype(np.int64)
    T = 1 + S0 + S1                           # self col + both windows

    pos2orig = np.full(cfg.NPAD, -1, dtype=np.int64)
    valid = orig2pos >= 0
    pos2orig[orig2pos[valid]] = np.nonzero(valid)[0]

    # per-core idx / mask arrays
    IA = int(8 * S0.sum())
    IB = int(8 * S1.sum())
    MT = int(T.sum())
    idxA = np.zeros((CORES, 128, IA), np.int16)
    idxB = np.zeros((CORES, 128, IB), np.int16)
    mneg = np.full((CORES, 128, MT), MASK_NEG, np.float32)

    for k in range(CORES):
        c = per_core[k]
        # bucket edges by local dst position
        pos_of_dst = orig2pos[c["edges_dst_local"] + PER * k] - PERP * k
        spos = orig2pos[c["edges_src"]]
        isw0 = spos < cfg.WIN
        # per destination-position lists of src positions
        bucket0 = [[] for _ in range(PERP)]
        bucket1 = [[] for _ in range(PERP)]
        for p, sp, w in zip(pos_of_dst, spos, isw0):
            (bucket0 if w else bucket1)[p].append(sp)
        aoff = boff = moff = 0
        for b in range(NB):
            s0, s1, t = int(S0[b]), int(S1[b]), int(T[b])
            flatA = np.zeros(128 * s0, np.int64)
            flatB = np.zeros(128 * s1, np.int64)
            for p in range(128):
                g = 128 * b + p
                l0, l1 = bucket0[g], bucket1[g]
                for s_i, sp in enumerate(l0):
                    flatA[s_i * 128 + p] = sp
                for s_i, sp in enumerate(l1):
                    flatB[s_i * 128 + p] = sp - cfg.WIN
                mneg[k, p, moff] = 0.0                      # self col
                mneg[k, p, moff + 1: moff + 1 + len(l0)] = 0.0
                mneg[k, p, moff + 1 + s0: moff + 1 + s0 + len(l1)] = 0.0
            # wrap indices into [128, n/16] int16 (16-row layout, replicated)
            for nfl, arr, off in ((s0, flatA, aoff), (s1, flatB, boff)):
                if nfl == 0:
                    continue
                cols = 8 * nfl
                wrapped = arr.reshape(cols, 16).T.astype(np.int16)  # [16, cols]
                tgt = idxA if arr is flatA else idxB
                tgt[k, :, off:off + cols] = np.tile(wrapped, (8, 1))
            aoff += 8 * s0
            boff += 8 * s1
            moff += t

    stats = {
        "real_edges": int(sum(len(c["edges_src"]) for c in per_core)) + N,
        "padded_edges": int((T.sum()) * 128 * CORES),
    }
    return {
        "S0": S0, "S1": S1, "T": T, "IA": IA, "IB": IB, "MT": MT,
        "idxA": idxA, "idxB": idxB, "mneg": mneg,
        "orig2pos": orig2pos, "pos2orig": pos2orig, "stats": stats,
    }


# ---------------------------------------------------------------------------
# Bass program
# ---------------------------------------------------------------------------

def build_program(cfg, topo):
    import concourse.bacc as bacc
    import concourse.mybir as mybir
    import concourse.tile as tile

    dt = mybir.dt
    S0, S1, T = topo["S0"], topo["S1"], topo["T"]
    IA, IB, MT = topo["IA"], topo["IB"], topo["MT"]
    NPAD, PERP, WIN, NB, NT = cfg.NPAD, cfg.PERP, cfg.WIN, cfg.NB, cfg.NT
    NTO = PERP // 128                           # own node tiles (== NB)

    CCX = F * PERP                              # bf16 elems of x2T section
    CCA = PERP                                  # f32 elems of a_src2 section
    CCTOT = CCX + 2 * CCA                       # bf16 elems per rank

    nc = bacc.Bacc("TRN2", target_bir_lowering=False, debug=False,
                   enable_asserts=False, num_devices=CORES,
                   num_swdge_queues=NQUEUES)

    # --- kernel I/O ---
    xTg = nc.dram_tensor("xTg", [F, NPAD], dt.bfloat16, kind="ExternalInput")
    xTo = nc.dram_tensor("xTo", [F, PERP], dt.bfloat16, kind="ExternalInput")
    W1e = nc.dram_tensor("W1e", [F, 130], dt.bfloat16, kind="ExternalInput")
    W2e = nc.dram_tensor("W2e", [F, 130], dt.bfloat16, kind="ExternalInput")
    idxA_d = nc.dram_tensor("idxA", [128, max(IA, 16)], dt.int16, kind="ExternalInput")
    idxB_d = nc.dram_tensor("idxB", [128, max(IB, 16)], dt.int16, kind="ExternalInput")
    mneg_d = nc.dram_tensor("mneg", [128, MT], dt.float32, kind="ExternalInput")
    b1r_d = nc.dram_tensor("b1r", [128, F], dt.float32, kind="ExternalInput")
    b2r_d = nc.dram_tensor("b2r", [128, F], dt.float32, kind="ExternalInput")
    eye_d = nc.dram_tensor("eye", [128, 128], dt.float32, kind="ExternalInput")
    out_d = nc.dram_tensor("out", [PERP, F], dt.float32, kind="ExternalOutput")

    # --- internal DRAM ---
    h1x = nc.dram_tensor("h1x", [NPAD, ROW], dt.bfloat16)
    h1o = nc.dram_tensor("h1o", [PERP, ROW], dt.bfloat16)
    h2x = nc.dram_tensor("h2x", [NPAD, ROW], dt.bfloat16)
    h2o = nc.dram_tensor("h2o", [PERP, ROW], dt.bfloat16)
    cc_in = nc.dram_tensor("cc_in", [CCTOT], dt.bfloat16)
    cc_out = nc.dram_tensor("cc_out", [CORES, CCTOT], dt.bfloat16,
                            addr_space="Shared")

    f32r = dt.float32r

    with tile.TileContext(nc) as tc, ExitStack() as ctx:
        P = ctx.enter_context(tc.tile_pool(name="persist", bufs=1))

        # persistent SBUF
        idxA_s = P.tile([128, max(IA, 16)], dt.int16)
        idxB_s = P.tile([128, max(IB, 16)], dt.int16)
        mneg_s = P.tile([128, MT], dt.float32)
        W1e_s = P.tile([F, 130], dt.bfloat16)
        W2e_s = P.tile([F, 130], dt.bfloat16)
        b1r_s = P.tile([128, F], dt.float32)
        b2r_s = P.tile([128, F], dt.float32)
        eye_s = P.tile([128, 128], dt.float32)
        adst1 = P.tile([128, NB], dt.float32)
        adst2 = P.tile([128, NB], dt.float32)
        asrc2 = P.tile([128, NB], dt.float32)
        x2Tb = P.tile([F, PERP], dt.bfloat16)

        nc.sync.dma_start(idxA_s[:], idxA_d[:])
        nc.sync.dma_start(idxB_s[:], idxB_d[:])
        nc.sync.dma_start(mneg_s[:], mneg_d[:])
        nc.sync.dma_start(W1e_s[:], W1e[:])
        nc.sync.dma_start(W2e_s[:], W2e[:])
        nc.sync.dma_start(b1r_s[:], b1r_d[:])
        nc.sync.dma_start(b2r_s[:], b2r_d[:])
        nc.sync.dma_start(eye_s[:], eye_d[:])

        def h_phase(xsrc_kind, hx_dram, ho_dram, We_s, adst_s, layer):
            """Construct h_ext rows for all nodes (global) + own nodes."""
            with tc.tile_pool(name=f"hx{layer}", bufs=4) as hp, \
                 tc.tile_pool(name=f"hps{layer}", bufs=3, space="PSUM") as pp:
                # global tiles
                for t in range(NT):
                    xt = hp.tile([F, 128], dt.bfloat16, tag="xt")
                    if xsrc_kind == "xTg":
                        nc.sync.dma_start(xt[:], xTg[:, 128 * t:128 * (t + 1)])
                        ps = pp.tile([128, 130], dt.float32, tag="ps")
                        nc.tensor.matmul(ps[:], xt[:], We_s[:])
                        asrc_src = ps[:, 128:129]
                    else:  # layer 2: bf16 shards + exchanged f32 a_src
                        r, cb = t // NB, t % NB
                        x2sec = cc_out[r, 0:CCX].rearrange("(f n) -> f n", f=F)
                        nc.sync.dma_start(
                            xt[:], x2sec[:, 128 * cb:128 * (cb + 1)])
                        at = hp.tile([128, 1], dt.float32, tag="at")
                        asec = cc_out[r, CCX:CCX + 2 * CCA].bitcast(dt.float32)
                        nc.sync.dma_start(
                            at[:], asec[128 * cb:128 * (cb + 1)].rearrange("(n o) -> n o", o=1))
                        ps = pp.tile([128, 130], dt.float32, tag="ps")
                        nc.tensor.matmul(ps[:], xt[:], We_s[:])
                        asrc_src = at[:]
                    hx = hp.tile([128, 132], dt.bfloat16, tag="hx")
                    nc.scalar.activation(hx[:, 0:F], ps[:, 0:F],
                                         mybir.ActivationFunctionType.Copy)
                    nc.vector.memset(hx[:, 128:130], 0)
                    h32 = hx[:].bitcast(dt.float32)
                    nc.vector.tensor_copy(h32[:, ACOL:ACOL + 1], asrc_src)
                    nc.sync.dma_start(hx_dram[128 * t:128 * (t + 1), 0:132], hx[:])
                # own tiles (f32 source, also extract a_dst)
                for b in range(NTO):
                    if xsrc_kind == "xTg":
                        xt = hp.tile([F, 128], dt.bfloat16, tag="xto")
                        nc.sync.dma_start(xt[:], xTo[:, 128 * b:128 * (b + 1)])
                        lhs = xt[:]
                    else:
                        lhs = x2Tb[:, 128 * b:128 * (b + 1)]
                    ps = pp.tile([128, 130], dt.float32, tag="pso")
                    nc.tensor.matmul(ps[:], lhs,
                                     We_s[:])
                    hx = hp.tile([128, 132], dt.bfloat16, tag="hxo")
                    nc.scalar.activation(hx[:, 0:F], ps[:, 0:F],
                                         mybir.ActivationFunctionType.Copy)
                    nc.vector.memset(hx[:, 128:130], 0)
                    h32 = hx[:].bitcast(dt.float32)
                    nc.vector.tensor_copy(h32[:, ACOL:ACOL + 1], ps[:, 128:129])
                    nc.vector.tensor_copy(adst_s[:, b:b + 1], ps[:, 129:130])
                    nc.sync.dma_start(ho_dram[128 * b:128 * (b + 1), 0:132], hx[:])

        def edge_phase(hx_dram, ho_dram, adst_s, br_s, layer):
            """Per-block gather + softmax + weighted aggregation."""
            edge_phase.q = getattr(edge_phase, "q", 0)
            aoff = boff = moff = 0
            with tc.tile_pool(name=f"G{layer}", bufs=2) as gp, \
                 tc.tile_pool(name=f"wg{layer}", bufs=2) as wp, \
                 tc.tile_pool(name=f"sc{layer}", bufs=3) as sp, \
                 tc.tile_pool(name=f"ag{layer}", bufs=2) as apool, \
                 tc.tile_pool(name=f"ps{layer}", bufs=3, space="PSUM") as pp:
                for b in range(NB):
                    s0, s1, t = int(S0[b]), int(S1[b]), int(T[b])
                    G = gp.tile([128, t, ROW], dt.bfloat16, tag="G")
                    nc.sync.dma_start(G[:, 0, :],
                                      ho_dram[128 * b:128 * (b + 1), :])
                    for c0 in range(0, s0, GCHUNK):
                        cn = min(GCHUNK, s0 - c0)
                        nc.gpsimd.dma_gather(
                            G[:, 1 + c0:1 + c0 + cn, :], hx_dram[0:WIN, :],
                            idxA_s[:, aoff + 8 * c0:aoff + 8 * (c0 + cn)],
                            128 * cn, 128 * cn, ROW,
                            queue_num=edge_phase.q % NQUEUES)
                        edge_phase.q += 1
                    for c0 in range(0, s1, GCHUNK):
                        cn = min(GCHUNK, s1 - c0)
                        nc.gpsimd.dma_gather(
                            G[:, 1 + s0 + c0:1 + s0 + c0 + cn, :],
                            hx_dram[WIN:NPAD, :],
                            idxB_s[:, boff + 8 * c0:boff + 8 * (c0 + cn)],
                            128 * cn, 128 * cn, ROW,
                            queue_num=edge_phase.q % NQUEUES)
                        edge_phase.q += 1
                    G32 = G[:].bitcast(dt.float32)   # [128, t, ROW//2]
                    E = sp.tile([128, t], dt.float32, tag="E")
                    # E = (a_src + a_dst) + mask(0 | -30000)
                    nc.vector.scalar_tensor_tensor(
                        E[:], G32[:, :, ACOL], adst_s[:, b:b + 1],
                        mneg_s[:, moff:moff + t],
                        mybir.AluOpType.add, mybir.AluOpType.add)
                    EL = sp.tile([128, t], dt.float32, tag="EL")
                    # leaky relu: max(0.2*E, E)
                    nc.vector.scalar_tensor_tensor(
                        EL[:], E[:], NEG_SLOPE, E[:],
                        mybir.AluOpType.mult, mybir.AluOpType.max)
                    EX = sp.tile([128, t], dt.float32, tag="EX")
                    den = sp.tile([128, 1], dt.float32, tag="den")
                    nc.scalar.activation(EX[:], EL[:],
                                         mybir.ActivationFunctionType.Exp,
                                         accum_out=den[:])
                    rec = sp.tile([128, 1], dt.float32, tag="rec")
                    nc.vector.reciprocal(rec[:], den[:])
                    wG = wp.tile([128, t, F], dt.bfloat16, tag="wG")
                    nc.vector.tensor_tensor(
                        wG[:], G[:, :, 0:F],
                        EX[:].unsqueeze(2).broadcast_to([128, t, F]),
                        mybir.AluOpType.mult)
                    agg = apool.tile([128, F], dt.float32, tag="agg")
                    nc.vector.tensor_reduce(
                        agg[:], wG[:].transpose([0, 2, 1]),
                        mybir.AxisListType.X, mybir.AluOpType.add)
                    o = apool.tile([128, F], dt.float32, tag="o")
                    nc.vector.scalar_tensor_tensor(
                        o[:], agg[:], rec[:, 0:1], br_s[:],
                        mybir.AluOpType.mult, mybir.AluOpType.add)
                    o2 = apool.tile([128, F], dt.float32, tag="o2")
                    nc.scalar.activation(o2[:], o[:],
                                         mybir.ActivationFunctionType.Relu)
                    if layer == 1:
                        # x2 block: transpose into x2T (bf16)
                        psT = pp.tile([128, 128], dt.float32, tag="psT")
                        nc.tensor.transpose(psT[:], o2[:], eye_s[:])
                        cols = slice(128 * b, 128 * (b + 1))
                        nc.vector.tensor_copy(x2Tb[:, cols], psT[:])
                        ps2 = pp.tile([128, 2], dt.float32, tag="ps2")
                        nc.tensor.matmul(ps2[:],
                                         x2Tb[:, cols],
                                         W2e_s[:, 128:130])
                        nc.vector.tensor_copy(asrc2[:, b:b + 1], ps2[:, 0:1])
                        nc.vector.tensor_copy(adst2[:, b:b + 1], ps2[:, 1:2])
                    else:
                        nc.sync.dma_start(out_d[128 * b:128 * (b + 1), :], o2[:])
                    aoff += 8 * s0
                    boff += 8 * s1
                    moff += t

        # ---- layer 1 ----
        h_phase("xTg", h1x, h1o, W1e_s, adst1, layer=1)
        edge_phase(h1x, h1o, adst1, b1r_s, layer=1)

        # ---- exchange ----
        ccx = cc_in[0:CCX].rearrange("(f n) -> f n", f=F)
        nc.sync.dma_start(ccx, x2Tb[:])
        cca = cc_in[CCX:CCX + 2 * CCA].bitcast(dt.float32)
        cca_pb = cca.rearrange("(b p) -> p b", p=128)
        nc.sync.dma_start(cca_pb, asrc2[:])
        nc.gpsimd.collective_compute(
            "AllGather", mybir.AluOpType.bypass,
            replica_groups=[list(range(CORES))],
            ins=[cc_in[:].opt()], outs=[cc_out[:].opt()])

        # ---- layer 2 ----
        h_phase("cc", h2x, h2o, W2e_s, adst2, layer=2)
        edge_phase(h2x, h2o, adst2, b2r_s, layer=2)

    nc.compile()
    return nc


# ---------------------------------------------------------------------------
# Host orchestration
# ---------------------------------------------------------------------------

def make_inputs(cfg, topo, x, W1, as1, ad1, b1, W2, as2, ad2, b2):
    N, NPAD, PERP = cfg.N, cfg.NPAD, cfg.PERP
    bf16 = ml_dtypes.bfloat16
    pos2orig = topo["pos2orig"]

    xT = np.zeros((F, NPAD), bf16)
    valid = pos2orig >= 0
    xT[:, valid] = np.asarray(x, np.float32)[pos2orig[valid]].T.astype(bf16)

    def wext(W, a_s, a_d):
        W = np.asarray(W, np.float64)
        return np.concatenate(
            [W, (W @ np.asarray(a_s, np.float64))[:, None],
             (W @ np.asarray(a_d, np.float64))[:, None]], axis=1
        ).astype(bf16)

    W1e = wext(W1, as1, ad1)
    W2e = wext(W2, as2, ad2)
    b1r = np.tile(np.asarray(b1, np.float32)[None, :], (128, 1))
    b2r = np.tile(np.asarray(b2, np.float32)[None, :], (128, 1))
    eye = np.eye(128, dtype=np.float32)

    in_maps = []
    for k in range(CORES):
        in_maps.append({
            "xTg": xT,
            "xTo": np.ascontiguousarray(xT[:, PERP * k:PERP * (k + 1)]),
            "W1e": W1e, "W2e": W2e,
            "idxA": topo["idxA"][k] if topo["IA"] else np.zeros((128, 16), np.int16),
            "idxB": topo["idxB"][k] if topo["IB"] else np.zeros((128, 16), np.int16),
            "mneg": topo["mneg"][k],
            "b1r": b1r, "b2r": b2r, "eye": eye,
        })
    return in_maps


_CACHE = {}


def _get_program(cfg, edge_index):
    key = hashlib.sha1(np.ascontiguousarray(edge_index).tobytes()).hexdigest()
    if key not in _CACHE:
        topo = build_topology(cfg, edge_index)
        nc = build_program(cfg, topo)
        _CACHE[key] = (topo, nc)
    return _CACHE[key]


def run(cfg, inputs, trace=False):
    from concourse.bass_utils import run_bass_kernel_spmd

    topo, nc = _get_program(cfg, inputs["edge_index"])
    in_maps = make_inputs(
        cfg, topo, inputs["x"],
        inputs["W1"], inputs["att_src1"], inputs["att_dst1"], inputs["b1"],
        inputs["W2"], inputs["att_src2"], inputs["att_dst2"], inputs["b2"])
    res = run_bass_kernel_spmd(nc, in_maps, list(range(CORES)), trace=trace)

    full = np.zeros((cfg.N, F), np.float32)
    pos2orig = topo["pos2orig"]
    for k in range(CORES):
        o = np.asarray(res.results[k]["out"], np.float32)
        po = pos2orig[cfg.PERP * k:cfg.PERP * (k + 1)]
        m = po >= 0
        full[po[m]] = o[m]
    return full, res


def kernel(**inputs) -> np.ndarray:
    out, _ = run(FULL_CFG, inputs)
    return out
